# revision 71
# baseline (speedup 1.0000x reference)
"""MultiHeadCrossModalAttention TRN2 kernel (8 NeuronCores, self-contained).

Problem (hardcoded): B=4, S=2048, D=512, H=8, HD=64, fp32.
  Q = heads(mod1 @ Wq + bq); K/V/scale/shift = heads(mod2 @ W* + b*)
  K = K*scale+shift; V = V*scale+shift
  out = softmax(Q K^T / 8) V  -> concat heads -> @ Wo + bo

Sharding: core c handles batch b=c//2 and head-group g=c%2 (4 heads,
256 feature cols). The output projection is row-split over head groups,
so each core produces a partial [S, D] product; the host sums the two
partials per batch (exact fp32 add) to unshard.

v5 design notes (on top of v3):
- Inputs arrive HOST-TRANSPOSED ([feat, seq] bf16), so there are no
  on-device input transposes, drains, or staging tiles at all; the
  x^T column groups stream straight into SBUF d-block tiles by DMA.
- Engine placement tuned to the CoreSim cost model: plain bias
  epilogues (s/sh/q) on ACT via activation(Identity, bias=[128,1] AP);
  FiLM first-ops (scalar_tensor_tensor from psum) on DVE, second-ops
  on GPSIMD; memsets on GPSIMD; attention-output drain (oc) on ACT;
  reciprocal on DVE; broadcast+normalize multiplies on GPSIMD.
  GPSIMD cannot touch PSUM (BIR verifier) and has no divide.
- The softmax exp is the bottleneck: 256 half-tiles of [128,512] psum
  scores in a 4-deep one-bank psum ring; each k-tile's two head-halves
  can run on BOTH engines in parallel (hi0 on ACT table-exp -> fp8,
  hi1 on the DVE bitcast-exp when the k-tile is in DVE_KTS), tuned
  per chunk so ACT/DVE busy stay balanced.
- All independent work (projections, output projection, normalize) is
  emitted as small "filler" units (half projection chunks etc.) pumped
  between the score matmuls and the exp of each k-tile, so no in-order
  engine queue ever blocks head-of-line; attn-V matmuls trail one
  k-pair behind the scores inside the same chunk.
- Phase M streams chunk (0,0) AND the first 12 k-tiles of chunk (0,1)
  (attn-V deferred until the single psum accumulator frees) so the exp
  engines are fed while the r0 projections run; weight DMAs are spread
  across the ACT/SP HWDGE and GPSIMD SWDGE queues.
- attn-V: fp8 DoubleRow over k-tile pairs; a ones column in the V
  operand accumulates the softmax denominator in psum row 64.
- attn_norm of chunk n is pumped as a filler inside chunk n+1; the
  final chunk's normalize is strip-mined per output s-tile, with bias
  folded into a ones-row matmul and drains alternating ACT/DVE.
"""
import collections
import numpy as np
import concourse.mybir as mybir
import concourse.tile as tile
from concourse import bacc
from concourse.bass_utils import run_bass_kernel_spmd
from concourse.masks import make_identity
from contextlib import ExitStack

F32 = mybir.dt.float32
F32R = mybir.dt.float32r
BF16 = mybir.dt.bfloat16
F8 = mybir.dt.float8e4
U8 = mybir.dt.uint8
U32 = mybir.dt.uint32
AF = mybir.ActivationFunctionType
OP = mybir.AluOpType
DRm = mybir.MatmulPerfMode.DoubleRow

B, S, D, H = 4, 2048, 512, 8
HD = 64          # head dim
NG = 256         # feature cols per head-group (4 heads)
NH = 4           # heads per group
DB = D // 128    # 4 d-blocks
KT = S // 128    # 16 k-tiles
N_CORES = 8
C1 = 8 * 0.125 / np.log(2)   # bitcast-exp scale
C2 = 55.54                   # bitcast-exp offset (round-convert tuned)

# DVE-exp k-tile assignment per (j, qc) chunk, tuned so ACT and DVE
# stay equally busy given their other work per phase.
ND = {3, 7, 11}        # kts whose hi1 exp stays on ACT
DVE_KTS = {
    (0, 0): set(range(16)) - {3, 7, 11},
    (0, 1): set(range(16)) - {3, 7},
    (0, 2): set(range(16)),
    (0, 3): set(range(16)) - {11},
    (1, 0): set(range(16)) - {3, 11},
    (1, 1): set(range(16)) - {3, 11},
    (1, 2): set(range(16)) - {3, 11},
    (1, 3): set(range(16)) - {13, 14, 15},
}


def build():
    nc = bacc.Bacc(None)
    x1 = nc.dram_tensor("x1", [D, S], BF16, kind="ExternalInput")
    x2 = nc.dram_tensor("x2", [D, S], BF16, kind="ExternalInput")
    w_in = {}
    for p in ("q", "k", "v", "s", "sh"):
        w_in[p] = nc.dram_tensor(f"w{p}", [D, NG], BF16, kind="ExternalInput")
    ball_in = nc.dram_tensor("ball", [5 * NG], F32, kind="ExternalInput")
    wo = nc.dram_tensor("wo", [NG, D], F32R, kind="ExternalInput")
    bo = nc.dram_tensor("bo", [D], F32, kind="ExternalInput")
    out = nc.dram_tensor("out", [S, D], F32, kind="ExternalOutput")

    with tile.TileContext(nc) as tc, ExitStack() as top:
        cst = top.enter_context(tc.tile_pool(name="cst", bufs=1))
        # PSUM pool, three tag rings:
        #   P: projections/V-transpose/out-proj [128,512] x2 (2 banks)
        #   S: attention scores [128,1024] x2             (4 banks)
        #   B: attention out + denom row [65,1024] x1     (2 banks)
        psp = top.enter_context(tc.tile_pool(name="psp", bufs=2, space="PSUM"))

        def psP(f_dim, dt=F32):
            return psp.tile([128, f_dim], dt, tag="P", name="psP",
                            padded_shape=[128, 512])

        rrow = cst.tile([1, 1024], mybir.dt.uint32, tag="rrow",
                        name="rrow")
        twos = cst.tile([1, 1024], F32, tag="twos", name="twos")
        ident = cst.tile([128, 128], F32, tag="ident", name="ident")
        make_identity(nc, ident)
        identr = cst.tile([128, 128], F32R, tag="identr", name="identr")
        nc.vector.tensor_copy(identr, ident)
        identb = cst.tile([128, 128], BF16, tag="identb", name="identb")
        nc.vector.tensor_copy(identb, ident)

        # persistent activations
        actp = top.enter_context(tc.tile_pool(name="actp", bufs=1))
        # transposed bf16 inputs [feat, seq], all four 128-row d-blocks
        # side by side in one tile so a column group loads as ONE DMA
        x1t_b = actp.tile([128, DB * S], BF16, tag="x1t", name="x1t")
        x2t_b = actp.tile([128, DB * S], BF16, tag="x2t", name="x2t")
        x1t_v = x1t_b.rearrange("p (d s) -> p d s", d=DB)
        x2t_v = x2t_b.rearrange("p (d s) -> p d s", d=DB)
        x1t = [x1t_v[:, d, :] for d in range(DB)]
        x2t = [x2t_v[:, d, :] for d in range(DB)]
        # fp8 Q / K-film, [128, 2*S]: first S cols data, second S zeros
        # (zero halves make the DoubleRow score matmul contract 64 real
        # features + 64 zeros)
        Qb8 = [actp.tile([128, 2 * S], F8, tag=f"Qb8{r}", name=f"Qb8{r}")
               for r in range(2)]
        Kb8 = [actp.tile([128, 2 * S], F8, tag=f"Kb8{r}", name=f"Kb8{r}")
               for r in range(2)]

        def emit_zero_halves():
            for t in Qb8 + Kb8:
                nc.vector.memset(t[:, S:2 * S].bitcast(U32), 0)
        At = [actp.tile([128, S], F32R, tag=f"At{r}", name=f"At{r}")
              for r in range(2)]

        with tc.tile_pool(name="vgp", bufs=1) as vgp, \
             tc.tile_pool(name="ptp", bufs=12) as ptp, \
             tc.tile_pool(name="dnp", bufs=2) as dnp, \
             tc.tile_pool(name="osb", bufs=4) as osb:
            vaug = []
            for h in range(NH):
                vt = vgp.tile([128, KT * 80], F8, tag=f"vg{h}", name=f"vg{h}")
                vaug.append(vt)
            # zero halves + denominator ones columns FIRST, on the
            # otherwise-idle DVE queue: they must land before the first
            # score / attn-V matmuls read them, with margin, on the
            # very first (cold) invocation
            emit_zero_halves()
            for vt_ in vaug:
                nc.vector.memset(
                    vt_.rearrange("p (k c) -> p k c", c=80)[:, :, 64:65],
                    1.0)

            # ---- filler queue: independent work pumped into the gaps
            # of the score->exp pipeline (emitted between a k-tile's
            # score matmuls and its exp so no engine queue blocks).
            fillers = collections.deque()

            def pump(n=1):
                for _ in range(n):
                    if fillers:
                        fillers.popleft()()

            def pump_all():
                while fillers:
                    fillers.popleft()()

            def attn_norm(j, qc, o_ps):
                # ACT drains the psum accumulator; the reciprocal runs
                # entirely on GPSIMD as magic-constant seed + one Newton
                # step (|err| < 0.26%), then broadcast + multiplies.
                q_sl = slice(qc * 512, (qc + 1) * 512)
                oc = dnp.tile([65, 1024], F32, tag="oc", name="oc", bufs=2)
                bc = dnp.tile([64, 1024], F32, tag="bc", name="bc", bufs=1)
                nt = dnp.tile([1, 1024], F32, tag="nt", name="nt", bufs=1)
                nc.scalar.activation(oc, o_ps, AF.Copy)
                dn = dnp.tile([1, 1024], F32, tag="dn", name="dn", bufs=1)
                nc.gpsimd.tensor_copy(dn, oc[64:65, :])
                nc.gpsimd.tensor_tensor(bc[0:1, :].bitcast(U32), rrow,
                                        dn.bitcast(U32), op=OP.subtract)
                nc.gpsimd.tensor_tensor(nt, dn, bc[0:1, :], op=OP.mult)
                nc.gpsimd.tensor_tensor(nt, twos, nt, op=OP.subtract)
                nc.gpsimd.tensor_tensor(bc[0:1, :], bc[0:1, :], nt,
                                        op=OP.mult)
                nc.gpsimd.partition_broadcast(bc, bc[0:1, :])
                for hi in range(2):
                    nc.gpsimd.tensor_tensor(
                        At[j][64 * hi:64 * hi + 64, q_sl],
                        oc[0:64, hi * 512:(hi + 1) * 512],
                        bc[:, hi * 512:(hi + 1) * 512], op=OP.mult)

            with tc.tile_pool(name="fp1", bufs=1) as fp1, \
                 tc.tile_pool(name="wp", bufs=1) as wp:

                def proj_chunk(wts_p, src, r, copy_out, col, split=None):
                    ps = psP(col.stop - col.start)

                    def half(h):
                        for d in ((0, 1) if h == 0 else (2, 3)):
                            nc.tensor.matmul(
                                ps, wts_p[d][:, r * 128:(r + 1) * 128],
                                src[d][:, col], start=(d == 0),
                                stop=(d == DB - 1))
                        if h == 1:
                            copy_out(ps, col)
                    if split is None:
                        half(0)
                        half(1)
                    else:
                        split.append(lambda: half(0))
                        split.append(lambda: half(1))

                # ---- constants on the gpsimd SWDGE queue, first-use order
                ball = cst.tile([128, 10], F32, tag="ball", name="ball")
                nc.gpsimd.dma_start(
                    ball, ball_in[:].rearrange("(c p) -> p c", p=128))
                border = ("s", "k", "sh", "v", "q")
                bias = {}
                for pi, p in enumerate(border):
                    for r in range(2):
                        bias[(p, r)] = ball[:, 2 * pi + r:2 * pi + r + 1]
                wts = {}
                weng = {"s": nc.scalar, "k": nc.scalar, "sh": nc.gpsimd,
                        "v": nc.gpsimd, "q": nc.gpsimd}
                for p in border:
                    wt = wp.tile([128, DB * NG], BF16, tag=f"w{p}",
                                 name=f"w{p}")
                    weng[p].dma_start(
                        wt, w_in[p][:, :].rearrange("(d p) n -> p d n", d=DB))
                    wtv = wt.rearrange("p (d n) -> p d n", d=DB)
                    wts[p] = [wtv[:, d, :] for d in range(DB)]
                nc.gpsimd.memset(rrow, 0x7EF311C3)
                nc.gpsimd.memset(twos, 2.0)
                wo_t = []
                bo_bc = cst.tile([128, D], F32, tag="bo_bc", name="bo_bc")
                bo_row = cst.tile([1, D], F32, tag="bo_row", name="bo_row")
                bo_row_b = cst.tile([1, D], BF16, tag="bo_row_b",
                                    name="bo_row_b")
                ones_row = cst.tile([1, 128], BF16, tag="ones_row",
                                    name="ones_row")
                nc.gpsimd.memset(ones_row, 1.0)

                def load_out_consts():
                    for r in range(2):
                        t = cst.tile([128, D], F32R, tag=f"wo{r}",
                                     name=f"wo{r}")
                        nc.gpsimd.dma_start(t, wo[r * 128:(r + 1) * 128, :])
                        wo_t.append(t)
                    nc.gpsimd.dma_start(
                        bo_row, bo[:].rearrange("(o n) -> o n", o=1))
                    nc.gpsimd.tensor_copy(bo_row_b, bo_row)

                def emit_out_st(st, tail=False):
                    """Output projection s-tile (row-split partial)+bias.
                    Tail tiles fold the bias in as a ones-row matmul and
                    drain on ACT (idle at the tail) instead of DVE."""
                    op_ps = psP(512)
                    for r in range(2):
                        nc.tensor.matmul(
                            op_ps, At[r][:, st * 128:(st + 1) * 128],
                            wo_t[r], start=(r == 0), stop=not tail)
                    ot = osb.tile([128, D], F32, tag="ot", name="ot")
                    if tail:
                        nc.tensor.matmul(op_ps, ones_row, bo_row_b,
                                         start=False, stop=True)
                        nc.scalar.activation(ot, op_ps, AF.Copy)
                    else:
                        nc.vector.tensor_tensor(ot, op_ps, bo_bc, op=OP.add)
                    nc.sync.dma_start(out[st * 128:(st + 1) * 128, :], ot)

                Sb = fp1.tile([128, S], F32, tag="Sb0", name="Sb0")
                Shb = fp1.tile([128, S], F32, tag="Shb0", name="Shb0")
                Sb1 = fp1.tile([128, S], F32, tag="Sb1", name="Sb1")
                Shb1 = fp1.tile([128, S], F32, tag="Shb1", name="Shb1")
                SBr = (Sb, Sb1)
                SHr = (Shb, Shb1)

                def bias_act(dst_fn, p, r):
                    def cp(ps, col):
                        nc.scalar.activation(dst_fn(col), ps, AF.Identity,
                                             bias=bias[(p, r)])
                    return cp

                def film_into(dst_fn, p, r):
                    def cp(ps, col):
                        w = col.stop - col.start
                        t1 = fp1.tile([128, 512], F32, tag="t1",
                                      name="t1", bufs=3)[:, 0:w]
                        nc.vector.scalar_tensor_tensor(
                            t1, ps, bias[(p, r)], SBr[r][:, col],
                            op0=OP.add, op1=OP.mult)
                        nc.gpsimd.tensor_tensor(
                            dst_fn(col), t1, SHr[r][:, col], op=OP.add)
                    return cp

                def v_proj_part(r, sc, split=None):
                    """V proj + FiLM into a staging tile; returns it."""
                    col = slice(sc * 512, (sc + 1) * 512)
                    Vtc = fp1.tile([128, 512], BF16, tag="Vtc",
                                   name="Vtc", bufs=3)
                    proj_chunk(wts["v"], x2t, r,
                               film_into(lambda c: Vtc[:, :], "v", r), col,
                               split=split)
                    return Vtc

                def v_transpose_part(r, sc, Vtc):
                    """PE-transpose the FiLM'd V into vaug for k-tiles
                    4sc..4sc+3 of head pair r."""
                    pv = [psP(256, BF16), psP(256, BF16)]
                    for j4 in range(4):
                        for hi in range(2):
                            o = 64 * hi
                            nc.tensor.transpose(
                                pv[hi][:, j4 * 64:(j4 + 1) * 64],
                                Vtc[o:o + 64, j4 * 128:(j4 + 1) * 128],
                                identb[o:o + 64, o:o + 64])
                    for hi in range(2):
                        nc.vector.tensor_copy(
                            vaug[2 * r + hi].rearrange(
                                "p (k c) -> p k c", c=80
                            )[:, sc * 4:(sc + 1) * 4, 0:64],
                            pv[hi].bitcast(BF16).rearrange(
                                "p (k c) -> p k c", c=64))

                def v_chunk(r, sc, split=None):
                    Vtc = v_proj_part(r, sc, split=split)
                    if split is None:
                        v_transpose_part(r, sc, Vtc)
                    else:
                        split.append(
                            lambda: v_transpose_part(r, sc, Vtc))

                def s_proj(r, sc, split=None):
                    col = slice(sc * 512, (sc + 1) * 512)
                    proj_chunk(wts["s"], x2t, r,
                               bias_act(lambda c: SBr[r][:, c], "s", r),
                               col, split=split)

                def sh_proj(r, sc, split=None):
                    col = slice(sc * 512, (sc + 1) * 512)
                    proj_chunk(wts["sh"], x2t, r,
                               bias_act(lambda c: SHr[r][:, c], "sh", r),
                               col, split=split)

                def k_proj(r, sc, split=None):
                    col = slice(sc * 512, (sc + 1) * 512)
                    proj_chunk(wts["k"], x2t, r,
                               film_into(lambda c: Kb8[r][:, c], "k", r),
                               col, split=split)

                def q_proj(r, qc, split=None):
                    col = slice(qc * 512, (qc + 1) * 512)
                    proj_chunk(wts["q"], x1t, r,
                               bias_act(lambda c: Qb8[r][:, c], "q", r),
                               col, split=split)

                def load_xt(src_dram, xtv, sg, eng=None):
                    """One DMA for a 512-seq column group of the
                    (host-transposed) input, covering all d-blocks."""
                    cols = slice(sg * 512, (sg + 1) * 512)
                    (eng or nc.sync).dma_start(
                        xtv[:, :, cols],
                        src_dram[:, cols].rearrange(
                            "(d p) s -> p d s", d=DB))

                # ================= Phase M: merged stream =================
                # Per column group sg: x2+x1 transposes, r0 projections
                # (s/k/sh/q/v), score tiles for chunk (0,0) kt-slice
                # [4sg..4sg+4) and chunk (0,1) kt-slice [4(sg-1)..4sg)
                # (chunk (0,1) attn-V deferred until its accumulator
                # frees after norm(0,0)).
                o_ps = {(0, 0): psp.tile([65, 1024], F32, tag="B",
                                         name="o_ps00", bufs=1)}
                av_store = collections.defaultdict(list)

                def attn_slice(j, qc, pairs, dve_kts, do_pump=True):
                    """Scores + exp for k-pairs; avs go to av_store."""
                    q8v = Qb8[j].rearrange("p (two n) -> p two n", two=2)
                    k8v = Kb8[j].rearrange("p (two n) -> p two n", two=2)
                    for m in pairs:
                        pt = ptp.tile([128, 2048], F8, tag="pt", name="pt")
                        for i in range(2):
                            kt = 2 * m + i
                            stps = []
                            for hi in range(2):
                                stp = psp.tile([128, 512], F32, tag="S",
                                               name="stp", bufs=4)
                                o = 64 * hi
                                nc.tensor.matmul(
                                    stp,
                                    k8v[o:o + 64, :,
                                        kt * 128:(kt + 1) * 128],
                                    q8v[o:o + 64, :,
                                        qc * 512:(qc + 1) * 512],
                                    start=True, stop=True, perf_mode=DRm)
                                stps.append(stp)
                            if do_pump:
                                pump(1)
                            for hi in range(2):
                                dst = pt[:, i * 1024 + hi * 512:
                                         i * 1024 + (hi + 1) * 512]
                                if hi == 1 and kt in dve_kts:
                                    nc.vector.tensor_scalar(
                                        dst.bitcast(U8), stps[hi], C1, C2,
                                        op0=OP.mult, op1=OP.add)
                                else:
                                    nc.scalar.activation(dst, stps[hi],
                                                         AF.Exp,
                                                         scale=0.125)
                        ptv = pt.rearrange("p (i x) -> p i x", i=2)

                        def av(m=m, ptv=ptv, j=j, qc=qc):
                            o = o_ps[(j, qc)]
                            for hi in range(2):
                                nc.tensor.matmul(
                                    o[:, hi * 512:(hi + 1) * 512],
                                    vaug[2 * j + hi].rearrange(
                                        "p (k c) -> p k c", c=80
                                    )[:, 2 * m:2 * m + 2, 0:65],
                                    ptv[:, :, hi * 512:hi * 512 + 512],
                                    start=(m == 0), stop=(m == KT // 2 - 1),
                                    perf_mode=DRm)
                        av_store[(j, qc)].append(av)

                def attn_chunk(j, qc, pairs, dve_kts):
                    """Full chunk: slices with attn-V trailing one pair."""
                    avq = av_store[(j, qc)]
                    for m in pairs:
                        attn_slice(j, qc, (m,), dve_kts)
                        while len(avq) > 1:
                            avq.pop(0)()
                    pump_all()
                    while avq:
                        avq.pop(0)()

                load_xt(x2, x2t_v, 0)
                load_xt(x1, x1t_v, 0)
                for sg in range(4):
                    if sg < 3:
                        load_xt(x2, x2t_v, sg + 1)
                        load_xt(x1, x1t_v, sg + 1)
                    units = collections.deque()
                    kt1 = collections.deque()
                    if sg >= 1:
                        kt1.extend([2 * (sg - 1), 2 * sg - 1])

                    def u_pump(n):
                        for _ in range(n):
                            if units:
                                units.popleft()()

                    def kt_pump():
                        if kt1:
                            attn_slice(0, 1, (kt1.popleft(),),
                                       DVE_KTS[(0, 1)], do_pump=False)
                    if sg == 0:
                        # narrow-first startup: project the first 256
                        # K columns (and full Q) so the first score
                        # pair launches as early as possible
                        c0, c1 = slice(0, 256), slice(256, 512)
                        proj_chunk(wts["s"], x2t, 0,
                                   bias_act(lambda c: SBr[0][:, c],
                                            "s", 0), c0)
                        proj_chunk(wts["sh"], x2t, 0,
                                   bias_act(lambda c: SHr[0][:, c],
                                            "sh", 0), c0)
                        q_proj(0, 0)
                        proj_chunk(wts["k"], x2t, 0,
                                   film_into(lambda c: Kb8[0][:, c],
                                             "k", 0), c0)
                        attn_slice(0, 0, (0,), DVE_KTS[(0, 0)],
                                   do_pump=False)
                        proj_chunk(wts["s"], x2t, 0,
                                   bias_act(lambda c: SBr[0][:, c],
                                            "s", 0), c1)
                        proj_chunk(wts["sh"], x2t, 0,
                                   bias_act(lambda c: SHr[0][:, c],
                                            "sh", 0), c1)
                        proj_chunk(wts["k"], x2t, 0,
                                   film_into(lambda c: Kb8[0][:, c],
                                             "k", 0), c1)
                        attn_slice(0, 0, (1,), DVE_KTS[(0, 0)],
                                   do_pump=False)
                        v_chunk(0, 0)
                    else:
                        s_proj(0, sg, split=units)
                        k_proj(0, sg, split=units)
                        sh_proj(0, sg, split=units)
                        u_pump(2)
                        kt_pump()
                        u_pump(2)
                        kt_pump()
                        u_pump(2)
                        q_proj(0, sg, split=units)
                        v_chunk(0, sg, split=units)
                        for m in (2 * sg, 2 * sg + 1):
                            attn_slice(0, 0, (m,), DVE_KTS[(0, 0)],
                                       do_pump=False)
                            u_pump(3)
                        u_pump(8)
                    avq0 = av_store[(0, 0)]
                    while avq0:
                        avq0.pop(0)()
                load_out_consts()
                nc.gpsimd.partition_broadcast(bo_bc, bo_row)

                # ============ Phase A0: finish (0,1), then (0,2..3) ======
                norm_q = [(0, 0)]

                def reg_norm():
                    j, qc = norm_q.pop(0)
                    fillers.append(
                        lambda j=j, qc=qc: attn_norm(j, qc, o_ps[(j, qc)]))

                def reg_out_st(st):
                    op_ps = psP(512)

                    def u1():
                        nc.tensor.matmul(
                            op_ps, At[0][:, st * 128:(st + 1) * 128],
                            wo_t[0], start=True, stop=False)

                    def u2():
                        nc.tensor.matmul(
                            op_ps, At[1][:, st * 128:(st + 1) * 128],
                            wo_t[1], start=False, stop=True)
                        ot = osb.tile([128, D], F32, tag="ot", name="ot")
                        nc.vector.tensor_tensor(ot, op_ps, bo_bc,
                                                op=OP.add)
                        nc.sync.dma_start(out[st * 128:(st + 1) * 128, :],
                                          ot)
                    fillers.append(u1)
                    fillers.append(u2)

                def reg_col_group(r, sc):
                    s_proj(r, sc, split=fillers)
                    k_proj(r, sc, split=fillers)
                    sh_proj(r, sc, split=fillers)
                    v_chunk(r, sc, split=fillers)

                # finish chunk (0,1): norm(0,0) first, then the deferred
                # attn-V matmuls, then k-tiles 12-15
                reg_norm()
                o_ps[(0, 1)] = psp.tile([65, 1024], F32, tag="B",
                                        name="o_ps01", bufs=1)
                avq1 = av_store[(0, 1)]
                while avq1:
                    fillers.append(avq1.pop(0))
                attn_chunk(0, 1, (6, 7), DVE_KTS[(0, 1)])
                norm_q.append((0, 1))

                for qc in (2, 3):
                    reg_norm()
                    if qc == 2:
                        q_proj(1, 0, split=fillers)
                        q_proj(1, 1, split=fillers)
                        reg_col_group(1, 0)
                    else:
                        q_proj(1, 2, split=fillers)
                        q_proj(1, 3, split=fillers)
                        reg_col_group(1, 1)
                        reg_col_group(1, 2)
                    o_ps[(0, qc)] = psp.tile([65, 1024], F32, tag="B",
                                             name="o_ps", bufs=1)
                    attn_chunk(0, qc, range(KT // 2), DVE_KTS[(0, qc)])
                    norm_q.append((0, qc))

                # ============ Phase A1: chunks (1,0..3) ============
                out_sts = []
                for qc in range(4):
                    reg_norm()
                    if qc == 0:
                        reg_col_group(1, 3)
                    for st in out_sts:
                        reg_out_st(st)
                    out_sts = []
                    o_ps[(1, qc)] = psp.tile([65, 1024], F32, tag="B",
                                             name="o_ps", bufs=1)
                    attn_chunk(1, qc, range(KT // 2), DVE_KTS[(1, qc)])
                    norm_q.append((1, qc))
                    out_sts = list(range(qc * 4, qc * 4 + 4))

                # tail: last normalize in 128-col units, each output
                # s-tile emitted as soon as its At columns are ready
                j, qc = norm_q.pop(0)
                o_l = o_ps[(j, qc)]
                q0 = qc * 512
                oc = dnp.tile([65, 1024], F32, tag="oc", name="oc", bufs=2)
                bc = dnp.tile([64, 1024], F32, tag="bc", name="bc", bufs=1)
                nc.vector.reciprocal(bc[0:1, :], o_l[64:65, :])
                nc.scalar.activation(oc, o_l, AF.Copy)
                nc.gpsimd.partition_broadcast(bc, bc[0:1, :])
                for u in range(4):
                    for hi in range(2):
                        nc.gpsimd.tensor_tensor(
                            At[j][64 * hi:64 * hi + 64,
                                  q0 + u * 128:q0 + (u + 1) * 128],
                            oc[0:64, hi * 512 + u * 128:
                               hi * 512 + (u + 1) * 128],
                            bc[:, hi * 512 + u * 128:
                               hi * 512 + (u + 1) * 128], op=OP.mult)
                    if u < 3:
                        emit_out_st(out_sts[u], tail=(u % 2 == 0))
                    else:
                        # final tile: split drain+DMA in halves so the
                        # last DMA starts as early as possible
                        st = out_sts[u]
                        op_ps = psP(512)
                        for r in range(2):
                            nc.tensor.matmul(
                                op_ps, At[r][:, st * 128:(st + 1) * 128],
                                wo_t[r], start=(r == 0), stop=(r == 1))
                        ot = osb.tile([128, D], F32, tag="ot", name="ot")
                        for h in range(2):
                            cs = slice(h * 256, (h + 1) * 256)
                            nc.vector.tensor_tensor(
                                ot[:, cs], op_ps[:, cs], bo_bc[:, cs],
                                op=OP.add)
                            nc.sync.dma_start(
                                out[st * 128:(st + 1) * 128, cs],
                                ot[:, cs])

    nc.compile()
    return nc


_NC = None


def kernel(mod1_feat, mod2_feat, Wq, bq, Wk, bk, Wv, bv, Wo, bo, Ws, bs,
           Wsh, bsh):
    global _NC
    import ml_dtypes
    if _NC is None:
        _NC = build()
    bf = ml_dtypes.bfloat16
    zeros_bo = np.zeros_like(bo)
    x1b = [np.ascontiguousarray(mod1_feat[b].T).astype(bf)
           for b in range(B)]
    x2b = [np.ascontiguousarray(mod2_feat[b].T).astype(bf)
           for b in range(B)]
    in_maps = []
    for c in range(N_CORES):
        b, g = c // 2, c % 2
        cols = slice(g * NG, (g + 1) * NG)
        ball = np.ascontiguousarray(np.concatenate(
            [bs[cols], bk[cols], bsh[cols], bv[cols], bq[cols]]
        ).astype(np.float32))
        in_maps.append({
            "x1": x1b[b],
            "x2": x2b[b],
            "wq": np.ascontiguousarray(Wq[:, cols]).astype(bf),
            "wk": np.ascontiguousarray(Wk[:, cols]).astype(bf),
            "wv": np.ascontiguousarray(Wv[:, cols]).astype(bf),
            "ws": np.ascontiguousarray(Ws[:, cols]).astype(bf),
            "wsh": np.ascontiguousarray(Wsh[:, cols]).astype(bf),
            "ball": ball,
            "wo": np.ascontiguousarray(Wo[cols, :]),
            "bo": bo if g == 0 else zeros_bo,
        })
    res = run_bass_kernel_spmd(_NC, in_maps, list(range(N_CORES)))
    outs = [res.results[c]["out"] for c in range(N_CORES)]
    full = np.stack([outs[2 * b] + outs[2 * b + 1] for b in range(B)])
    return full.astype(np.float32)


# revision 73
# speedup vs baseline: 1.0007x; 1.0007x over previous
"""MultiHeadCrossModalAttention TRN2 kernel (8 NeuronCores, self-contained).

Problem (hardcoded): B=4, S=2048, D=512, H=8, HD=64, fp32.
  Q = heads(mod1 @ Wq + bq); K/V/scale/shift = heads(mod2 @ W* + b*)
  K = K*scale+shift; V = V*scale+shift
  out = softmax(Q K^T / 8) V  -> concat heads -> @ Wo + bo

Sharding: core c handles batch b=c//2 and head-group g=c%2 (4 heads,
256 feature cols). The output projection is row-split over head groups,
so each core produces a partial [S, D] product; the host sums the two
partials per batch (exact fp32 add) to unshard.

v5 design notes (on top of v3):
- Inputs arrive HOST-TRANSPOSED ([feat, seq] bf16), so there are no
  on-device input transposes, drains, or staging tiles at all; the
  x^T column groups stream straight into SBUF d-block tiles by DMA.
- Engine placement tuned to the CoreSim cost model: plain bias
  epilogues (s/sh/q) on ACT via activation(Identity, bias=[128,1] AP);
  FiLM first-ops (scalar_tensor_tensor from psum) on DVE, second-ops
  on GPSIMD; memsets on GPSIMD; attention-output drain (oc) on ACT;
  reciprocal on DVE; broadcast+normalize multiplies on GPSIMD.
  GPSIMD cannot touch PSUM (BIR verifier) and has no divide.
- The softmax exp is the bottleneck: 256 half-tiles of [128,512] psum
  scores in a 4-deep one-bank psum ring; each k-tile's two head-halves
  can run on BOTH engines in parallel (hi0 on ACT table-exp -> fp8,
  hi1 on the DVE bitcast-exp when the k-tile is in DVE_KTS), tuned
  per chunk so ACT/DVE busy stay balanced.
- All independent work (projections, output projection, normalize) is
  emitted as small "filler" units (half projection chunks etc.) pumped
  between the score matmuls and the exp of each k-tile, so no in-order
  engine queue ever blocks head-of-line; attn-V matmuls trail one
  k-pair behind the scores inside the same chunk.
- Phase M streams chunk (0,0) AND the first 12 k-tiles of chunk (0,1)
  (attn-V deferred until the single psum accumulator frees) so the exp
  engines are fed while the r0 projections run; weight DMAs are spread
  across the ACT/SP HWDGE and GPSIMD SWDGE queues.
- attn-V: fp8 DoubleRow over k-tile pairs; a ones column in the V
  operand accumulates the softmax denominator in psum row 64.
- attn_norm of chunk n is pumped as a filler inside chunk n+1; the
  final chunk's normalize is strip-mined per output s-tile, with bias
  folded into a ones-row matmul and drains alternating ACT/DVE.
"""
import collections
import numpy as np
import concourse.mybir as mybir
import concourse.tile as tile
from concourse import bacc
from concourse.bass_utils import run_bass_kernel_spmd
from concourse.masks import make_identity
from contextlib import ExitStack

F32 = mybir.dt.float32
F32R = mybir.dt.float32r
BF16 = mybir.dt.bfloat16
F8 = mybir.dt.float8e4
U8 = mybir.dt.uint8
U32 = mybir.dt.uint32
AF = mybir.ActivationFunctionType
OP = mybir.AluOpType
DRm = mybir.MatmulPerfMode.DoubleRow

B, S, D, H = 4, 2048, 512, 8
HD = 64          # head dim
NG = 256         # feature cols per head-group (4 heads)
NH = 4           # heads per group
DB = D // 128    # 4 d-blocks
KT = S // 128    # 16 k-tiles
N_CORES = 8
C1 = 8 * 0.125 / np.log(2)   # bitcast-exp scale
C2 = 55.54                   # bitcast-exp offset (round-convert tuned)

# DVE-exp k-tile assignment per (j, qc) chunk, tuned so ACT and DVE
# stay equally busy given their other work per phase.
ND = {3, 7, 11}        # kts whose hi1 exp stays on ACT
DVE_KTS = {
    (0, 0): set(range(16)) - {3, 7, 11},
    (0, 1): set(range(16)) - {3, 7},
    (0, 2): set(range(16)),
    (0, 3): set(range(16)) - {11},
    (1, 0): set(range(16)) - {3, 11},
    (1, 1): set(range(16)) - {3, 11},
    (1, 2): set(range(16)) - {3, 11},
    (1, 3): set(range(16)) - {13, 14, 15},
}


def build():
    nc = bacc.Bacc(None)
    x1 = nc.dram_tensor("x1", [D, S], BF16, kind="ExternalInput")
    x2 = nc.dram_tensor("x2", [D, S], BF16, kind="ExternalInput")
    w_in = {}
    for p in ("q", "k", "v", "s", "sh"):
        w_in[p] = nc.dram_tensor(f"w{p}", [D, NG], BF16, kind="ExternalInput")
    ball_in = nc.dram_tensor("ball", [5 * NG], F32, kind="ExternalInput")
    wo = nc.dram_tensor("wo", [NG, D], F32R, kind="ExternalInput")
    bo = nc.dram_tensor("bo", [D], F32, kind="ExternalInput")
    out = nc.dram_tensor("out", [S, D], F32, kind="ExternalOutput")

    with tile.TileContext(nc) as tc, ExitStack() as top:
        cst = top.enter_context(tc.tile_pool(name="cst", bufs=1))
        # PSUM pool, three tag rings:
        #   P: projections/V-transpose/out-proj [128,512] x2 (2 banks)
        #   S: attention scores [128,1024] x2             (4 banks)
        #   B: attention out + denom row [65,1024] x1     (2 banks)
        psp = top.enter_context(tc.tile_pool(name="psp", bufs=2, space="PSUM"))

        def psP(f_dim, dt=F32):
            return psp.tile([128, f_dim], dt, tag="P", name="psP",
                            padded_shape=[128, 512])

        rrow = cst.tile([1, 1024], mybir.dt.uint32, tag="rrow",
                        name="rrow")
        twos = cst.tile([1, 1024], F32, tag="twos", name="twos")
        ident = cst.tile([128, 128], F32, tag="ident", name="ident")
        make_identity(nc, ident)
        identr = cst.tile([128, 128], F32R, tag="identr", name="identr")
        nc.vector.tensor_copy(identr, ident)
        identb = cst.tile([128, 128], BF16, tag="identb", name="identb")
        nc.vector.tensor_copy(identb, ident)

        # persistent activations
        actp = top.enter_context(tc.tile_pool(name="actp", bufs=1))
        # transposed bf16 inputs [feat, seq], all four 128-row d-blocks
        # side by side in one tile so a column group loads as ONE DMA
        x1t_b = actp.tile([128, DB * S], BF16, tag="x1t", name="x1t")
        x2t_b = actp.tile([128, DB * S], BF16, tag="x2t", name="x2t")
        x1t_v = x1t_b.rearrange("p (d s) -> p d s", d=DB)
        x2t_v = x2t_b.rearrange("p (d s) -> p d s", d=DB)
        x1t = [x1t_v[:, d, :] for d in range(DB)]
        x2t = [x2t_v[:, d, :] for d in range(DB)]
        # fp8 Q / K-film, [128, 2*S]: first S cols data, second S zeros
        # (zero halves make the DoubleRow score matmul contract 64 real
        # features + 64 zeros)
        Qb8 = [actp.tile([128, 2 * S], F8, tag=f"Qb8{r}", name=f"Qb8{r}")
               for r in range(2)]
        Kb8 = [actp.tile([128, 2 * S], F8, tag=f"Kb8{r}", name=f"Kb8{r}")
               for r in range(2)]

        def emit_zero_halves():
            for t in Qb8 + Kb8:
                nc.vector.memset(t[:, S:2 * S].bitcast(U32), 0)
        At = [actp.tile([128, S], F32R, tag=f"At{r}", name=f"At{r}")
              for r in range(2)]

        with tc.tile_pool(name="vgp", bufs=1) as vgp, \
             tc.tile_pool(name="ptp", bufs=12) as ptp, \
             tc.tile_pool(name="dnp", bufs=2) as dnp, \
             tc.tile_pool(name="osb", bufs=4) as osb:
            vaug = []
            for h in range(NH):
                vt = vgp.tile([128, KT * 80], F8, tag=f"vg{h}", name=f"vg{h}")
                vaug.append(vt)
            # zero halves + denominator ones columns FIRST, on the
            # otherwise-idle DVE queue: they must land before the first
            # score / attn-V matmuls read them, with margin, on the
            # very first (cold) invocation
            emit_zero_halves()
            for vt_ in vaug:
                nc.vector.memset(
                    vt_.rearrange("p (k c) -> p k c", c=80)[:, :, 64:65],
                    1.0)

            # ---- filler queue: independent work pumped into the gaps
            # of the score->exp pipeline (emitted between a k-tile's
            # score matmuls and its exp so no engine queue blocks).
            fillers = collections.deque()

            def pump(n=1):
                for _ in range(n):
                    if fillers:
                        fillers.popleft()()

            def pump_all():
                while fillers:
                    fillers.popleft()()

            def attn_norm(j, qc, o_ps):
                # ACT drains the psum accumulator; the reciprocal runs
                # entirely on GPSIMD as magic-constant seed + one Newton
                # step (|err| < 0.26%), then broadcast + multiplies.
                q_sl = slice(qc * 512, (qc + 1) * 512)
                oc = dnp.tile([65, 1024], F32, tag="oc", name="oc", bufs=2)
                bc = dnp.tile([64, 1024], F32, tag="bc", name="bc", bufs=2)
                nt = dnp.tile([1, 1024], F32, tag="nt", name="nt", bufs=2)
                nc.scalar.activation(oc, o_ps, AF.Copy)
                dn = dnp.tile([1, 1024], F32, tag="dn", name="dn", bufs=2)
                nc.gpsimd.tensor_copy(dn, oc[64:65, :])
                nc.gpsimd.tensor_tensor(bc[0:1, :].bitcast(U32), rrow,
                                        dn.bitcast(U32), op=OP.subtract)
                nc.gpsimd.tensor_tensor(nt, dn, bc[0:1, :], op=OP.mult)
                nc.gpsimd.tensor_tensor(nt, twos, nt, op=OP.subtract)
                nc.gpsimd.tensor_tensor(bc[0:1, :], bc[0:1, :], nt,
                                        op=OP.mult)
                nc.gpsimd.partition_broadcast(bc, bc[0:1, :])
                for hi in range(2):
                    nc.gpsimd.tensor_tensor(
                        At[j][64 * hi:64 * hi + 64, q_sl],
                        oc[0:64, hi * 512:(hi + 1) * 512],
                        bc[:, hi * 512:(hi + 1) * 512], op=OP.mult)

            with tc.tile_pool(name="fp1", bufs=1) as fp1, \
                 tc.tile_pool(name="wp", bufs=1) as wp:

                def proj_chunk(wts_p, src, r, copy_out, col, split=None):
                    ps = psP(col.stop - col.start)

                    def half(h):
                        for d in ((0, 1) if h == 0 else (2, 3)):
                            nc.tensor.matmul(
                                ps, wts_p[d][:, r * 128:(r + 1) * 128],
                                src[d][:, col], start=(d == 0),
                                stop=(d == DB - 1))
                        if h == 1:
                            copy_out(ps, col)
                    if split is None:
                        half(0)
                        half(1)
                    else:
                        split.append(lambda: half(0))
                        split.append(lambda: half(1))

                # ---- constants on the gpsimd SWDGE queue, first-use order
                ball = cst.tile([128, 10], F32, tag="ball", name="ball")
                nc.gpsimd.dma_start(
                    ball, ball_in[:].rearrange("(c p) -> p c", p=128))
                border = ("s", "k", "sh", "v", "q")
                bias = {}
                for pi, p in enumerate(border):
                    for r in range(2):
                        bias[(p, r)] = ball[:, 2 * pi + r:2 * pi + r + 1]
                wts = {}
                weng = {"s": nc.scalar, "k": nc.scalar, "sh": nc.gpsimd,
                        "v": nc.gpsimd, "q": nc.gpsimd}
                for p in border:
                    wt = wp.tile([128, DB * NG], BF16, tag=f"w{p}",
                                 name=f"w{p}")
                    weng[p].dma_start(
                        wt, w_in[p][:, :].rearrange("(d p) n -> p d n", d=DB))
                    wtv = wt.rearrange("p (d n) -> p d n", d=DB)
                    wts[p] = [wtv[:, d, :] for d in range(DB)]
                nc.gpsimd.memset(rrow, 0x7EF311C3)
                nc.gpsimd.memset(twos, 2.0)
                wo_t = []
                bo_bc = cst.tile([128, D], F32, tag="bo_bc", name="bo_bc")
                bo_row = cst.tile([1, D], F32, tag="bo_row", name="bo_row")
                bo_row_b = cst.tile([1, D], BF16, tag="bo_row_b",
                                    name="bo_row_b")
                ones_row = cst.tile([1, 128], BF16, tag="ones_row",
                                    name="ones_row")
                nc.gpsimd.memset(ones_row, 1.0)

                def load_out_consts():
                    for r in range(2):
                        t = cst.tile([128, D], F32R, tag=f"wo{r}",
                                     name=f"wo{r}")
                        nc.gpsimd.dma_start(t, wo[r * 128:(r + 1) * 128, :])
                        wo_t.append(t)
                    nc.gpsimd.dma_start(
                        bo_row, bo[:].rearrange("(o n) -> o n", o=1))
                    nc.gpsimd.tensor_copy(bo_row_b, bo_row)

                def emit_out_st(st, tail=False):
                    """Output projection s-tile (row-split partial)+bias.
                    Tail tiles fold the bias in as a ones-row matmul and
                    drain on ACT (idle at the tail) instead of DVE."""
                    op_ps = psP(512)
                    for r in range(2):
                        nc.tensor.matmul(
                            op_ps, At[r][:, st * 128:(st + 1) * 128],
                            wo_t[r], start=(r == 0), stop=not tail)
                    ot = osb.tile([128, D], F32, tag="ot", name="ot")
                    if tail:
                        nc.tensor.matmul(op_ps, ones_row, bo_row_b,
                                         start=False, stop=True)
                        nc.scalar.activation(ot, op_ps, AF.Copy)
                    else:
                        nc.vector.tensor_tensor(ot, op_ps, bo_bc, op=OP.add)
                    nc.sync.dma_start(out[st * 128:(st + 1) * 128, :], ot)

                Sb = fp1.tile([128, S], F32, tag="Sb0", name="Sb0")
                Shb = fp1.tile([128, S], F32, tag="Shb0", name="Shb0")
                Sb1 = fp1.tile([128, S], F32, tag="Sb1", name="Sb1")
                Shb1 = fp1.tile([128, S], F32, tag="Shb1", name="Shb1")
                SBr = (Sb, Sb1)
                SHr = (Shb, Shb1)

                def bias_act(dst_fn, p, r):
                    def cp(ps, col):
                        nc.scalar.activation(dst_fn(col), ps, AF.Identity,
                                             bias=bias[(p, r)])
                    return cp

                def film_into(dst_fn, p, r):
                    def cp(ps, col):
                        w = col.stop - col.start
                        t1 = fp1.tile([128, 512], F32, tag="t1",
                                      name="t1", bufs=2)[:, 0:w]
                        nc.vector.scalar_tensor_tensor(
                            t1, ps, bias[(p, r)], SBr[r][:, col],
                            op0=OP.add, op1=OP.mult)
                        nc.gpsimd.tensor_tensor(
                            dst_fn(col), t1, SHr[r][:, col], op=OP.add)
                    return cp

                def v_proj_part(r, sc, split=None):
                    """V proj + FiLM into a staging tile; returns it."""
                    col = slice(sc * 512, (sc + 1) * 512)
                    Vtc = fp1.tile([128, 512], BF16, tag="Vtc",
                                   name="Vtc", bufs=2)
                    proj_chunk(wts["v"], x2t, r,
                               film_into(lambda c: Vtc[:, :], "v", r), col,
                               split=split)
                    return Vtc

                def v_transpose_part(r, sc, Vtc):
                    """PE-transpose the FiLM'd V into vaug for k-tiles
                    4sc..4sc+3 of head pair r."""
                    pv = [psP(256, BF16), psP(256, BF16)]
                    for j4 in range(4):
                        for hi in range(2):
                            o = 64 * hi
                            nc.tensor.transpose(
                                pv[hi][:, j4 * 64:(j4 + 1) * 64],
                                Vtc[o:o + 64, j4 * 128:(j4 + 1) * 128],
                                identb[o:o + 64, o:o + 64])
                    for hi in range(2):
                        nc.vector.tensor_copy(
                            vaug[2 * r + hi].rearrange(
                                "p (k c) -> p k c", c=80
                            )[:, sc * 4:(sc + 1) * 4, 0:64],
                            pv[hi].bitcast(BF16).rearrange(
                                "p (k c) -> p k c", c=64))

                def v_chunk(r, sc, split=None):
                    Vtc = v_proj_part(r, sc, split=split)
                    if split is None:
                        v_transpose_part(r, sc, Vtc)
                    else:
                        split.append(
                            lambda: v_transpose_part(r, sc, Vtc))

                def s_proj(r, sc, split=None):
                    col = slice(sc * 512, (sc + 1) * 512)
                    proj_chunk(wts["s"], x2t, r,
                               bias_act(lambda c: SBr[r][:, c], "s", r),
                               col, split=split)

                def sh_proj(r, sc, split=None):
                    col = slice(sc * 512, (sc + 1) * 512)
                    proj_chunk(wts["sh"], x2t, r,
                               bias_act(lambda c: SHr[r][:, c], "sh", r),
                               col, split=split)

                def k_proj(r, sc, split=None):
                    col = slice(sc * 512, (sc + 1) * 512)
                    proj_chunk(wts["k"], x2t, r,
                               film_into(lambda c: Kb8[r][:, c], "k", r),
                               col, split=split)

                def q_proj(r, qc, split=None):
                    col = slice(qc * 512, (qc + 1) * 512)
                    proj_chunk(wts["q"], x1t, r,
                               bias_act(lambda c: Qb8[r][:, c], "q", r),
                               col, split=split)

                def load_xt(src_dram, xtv, sg, eng=None):
                    """One DMA for a 512-seq column group of the
                    (host-transposed) input, covering all d-blocks."""
                    cols = slice(sg * 512, (sg + 1) * 512)
                    (eng or nc.sync).dma_start(
                        xtv[:, :, cols],
                        src_dram[:, cols].rearrange(
                            "(d p) s -> p d s", d=DB))

                # ================= Phase M: merged stream =================
                # Per column group sg: x2+x1 transposes, r0 projections
                # (s/k/sh/q/v), score tiles for chunk (0,0) kt-slice
                # [4sg..4sg+4) and chunk (0,1) kt-slice [4(sg-1)..4sg)
                # (chunk (0,1) attn-V deferred until its accumulator
                # frees after norm(0,0)).
                o_ps = {(0, 0): psp.tile([65, 1024], F32, tag="B",
                                         name="o_ps00", bufs=1)}
                av_store = collections.defaultdict(list)

                def attn_slice(j, qc, pairs, dve_kts, do_pump=True):
                    """Scores + exp for k-pairs; avs go to av_store."""
                    q8v = Qb8[j].rearrange("p (two n) -> p two n", two=2)
                    k8v = Kb8[j].rearrange("p (two n) -> p two n", two=2)
                    for m in pairs:
                        pt = ptp.tile([128, 2048], F8, tag="pt", name="pt")
                        for i in range(2):
                            kt = 2 * m + i
                            stps = []
                            for hi in range(2):
                                stp = psp.tile([128, 512], F32, tag="S",
                                               name="stp", bufs=4)
                                o = 64 * hi
                                nc.tensor.matmul(
                                    stp,
                                    k8v[o:o + 64, :,
                                        kt * 128:(kt + 1) * 128],
                                    q8v[o:o + 64, :,
                                        qc * 512:(qc + 1) * 512],
                                    start=True, stop=True, perf_mode=DRm)
                                stps.append(stp)
                            if do_pump:
                                pump(1)
                            for hi in range(2):
                                dst = pt[:, i * 1024 + hi * 512:
                                         i * 1024 + (hi + 1) * 512]
                                if hi == 1 and kt in dve_kts:
                                    nc.vector.tensor_scalar(
                                        dst.bitcast(U8), stps[hi], C1, C2,
                                        op0=OP.mult, op1=OP.add)
                                else:
                                    nc.scalar.activation(dst, stps[hi],
                                                         AF.Exp,
                                                         scale=0.125)
                        ptv = pt.rearrange("p (i x) -> p i x", i=2)

                        def av(m=m, ptv=ptv, j=j, qc=qc):
                            o = o_ps[(j, qc)]
                            for hi in range(2):
                                nc.tensor.matmul(
                                    o[:, hi * 512:(hi + 1) * 512],
                                    vaug[2 * j + hi].rearrange(
                                        "p (k c) -> p k c", c=80
                                    )[:, 2 * m:2 * m + 2, 0:65],
                                    ptv[:, :, hi * 512:hi * 512 + 512],
                                    start=(m == 0), stop=(m == KT // 2 - 1),
                                    perf_mode=DRm)
                        av_store[(j, qc)].append(av)

                def attn_chunk(j, qc, pairs, dve_kts):
                    """Full chunk: slices with attn-V trailing one pair."""
                    avq = av_store[(j, qc)]
                    for m in pairs:
                        attn_slice(j, qc, (m,), dve_kts)
                        while len(avq) > 1:
                            avq.pop(0)()
                    pump_all()
                    while avq:
                        avq.pop(0)()

                load_xt(x2, x2t_v, 0)
                load_xt(x1, x1t_v, 0)
                for sg in range(4):
                    if sg < 3:
                        load_xt(x2, x2t_v, sg + 1)
                        load_xt(x1, x1t_v, sg + 1)
                    units = collections.deque()
                    kt1 = collections.deque()
                    if sg >= 1:
                        kt1.extend([2 * (sg - 1), 2 * sg - 1])

                    def u_pump(n):
                        for _ in range(n):
                            if units:
                                units.popleft()()

                    def kt_pump():
                        if kt1:
                            attn_slice(0, 1, (kt1.popleft(),),
                                       DVE_KTS[(0, 1)], do_pump=False)
                    if sg == 0:
                        # narrow-first startup: project the first 256
                        # K columns (and full Q) so the first score
                        # pair launches as early as possible
                        c0, c1 = slice(0, 256), slice(256, 512)
                        proj_chunk(wts["s"], x2t, 0,
                                   bias_act(lambda c: SBr[0][:, c],
                                            "s", 0), c0)
                        proj_chunk(wts["sh"], x2t, 0,
                                   bias_act(lambda c: SHr[0][:, c],
                                            "sh", 0), c0)
                        q_proj(0, 0)
                        proj_chunk(wts["k"], x2t, 0,
                                   film_into(lambda c: Kb8[0][:, c],
                                             "k", 0), c0)
                        attn_slice(0, 0, (0,), DVE_KTS[(0, 0)],
                                   do_pump=False)
                        proj_chunk(wts["s"], x2t, 0,
                                   bias_act(lambda c: SBr[0][:, c],
                                            "s", 0), c1)
                        proj_chunk(wts["sh"], x2t, 0,
                                   bias_act(lambda c: SHr[0][:, c],
                                            "sh", 0), c1)
                        proj_chunk(wts["k"], x2t, 0,
                                   film_into(lambda c: Kb8[0][:, c],
                                             "k", 0), c1)
                        attn_slice(0, 0, (1,), DVE_KTS[(0, 0)],
                                   do_pump=False)
                        v_chunk(0, 0)
                    else:
                        s_proj(0, sg, split=units)
                        k_proj(0, sg, split=units)
                        sh_proj(0, sg, split=units)
                        u_pump(2)
                        kt_pump()
                        u_pump(2)
                        kt_pump()
                        u_pump(2)
                        q_proj(0, sg, split=units)
                        v_chunk(0, sg, split=units)
                        for m in (2 * sg, 2 * sg + 1):
                            attn_slice(0, 0, (m,), DVE_KTS[(0, 0)],
                                       do_pump=False)
                            u_pump(3)
                        u_pump(8)
                    avq0 = av_store[(0, 0)]
                    while avq0:
                        avq0.pop(0)()
                load_out_consts()
                nc.gpsimd.partition_broadcast(bo_bc, bo_row)

                # ============ Phase A0: finish (0,1), then (0,2..3) ======
                norm_q = [(0, 0)]

                def reg_norm():
                    j, qc = norm_q.pop(0)
                    fillers.append(
                        lambda j=j, qc=qc: attn_norm(j, qc, o_ps[(j, qc)]))

                def reg_out_st(st):
                    op_ps = psP(512)

                    def u1():
                        nc.tensor.matmul(
                            op_ps, At[0][:, st * 128:(st + 1) * 128],
                            wo_t[0], start=True, stop=False)

                    def u2():
                        nc.tensor.matmul(
                            op_ps, At[1][:, st * 128:(st + 1) * 128],
                            wo_t[1], start=False, stop=True)
                        ot = osb.tile([128, D], F32, tag="ot", name="ot")
                        nc.vector.tensor_tensor(ot, op_ps, bo_bc,
                                                op=OP.add)
                        nc.sync.dma_start(out[st * 128:(st + 1) * 128, :],
                                          ot)
                    fillers.append(u1)
                    fillers.append(u2)

                def reg_col_group(r, sc):
                    s_proj(r, sc, split=fillers)
                    k_proj(r, sc, split=fillers)
                    sh_proj(r, sc, split=fillers)
                    v_chunk(r, sc, split=fillers)

                # finish chunk (0,1): norm(0,0) first, then the deferred
                # attn-V matmuls, then k-tiles 12-15
                reg_norm()
                o_ps[(0, 1)] = psp.tile([65, 1024], F32, tag="B",
                                        name="o_ps01", bufs=1)
                avq1 = av_store[(0, 1)]
                while avq1:
                    fillers.append(avq1.pop(0))
                attn_chunk(0, 1, (6, 7), DVE_KTS[(0, 1)])
                norm_q.append((0, 1))

                for qc in (2, 3):
                    reg_norm()
                    if qc == 2:
                        q_proj(1, 0, split=fillers)
                        q_proj(1, 1, split=fillers)
                        reg_col_group(1, 0)
                    else:
                        q_proj(1, 2, split=fillers)
                        q_proj(1, 3, split=fillers)
                        reg_col_group(1, 1)
                        reg_col_group(1, 2)
                    o_ps[(0, qc)] = psp.tile([65, 1024], F32, tag="B",
                                             name="o_ps", bufs=1)
                    attn_chunk(0, qc, range(KT // 2), DVE_KTS[(0, qc)])
                    norm_q.append((0, qc))

                # ============ Phase A1: chunks (1,0..3) ============
                out_sts = []
                for qc in range(4):
                    reg_norm()
                    if qc == 0:
                        reg_col_group(1, 3)
                    for st in out_sts:
                        reg_out_st(st)
                    out_sts = []
                    o_ps[(1, qc)] = psp.tile([65, 1024], F32, tag="B",
                                             name="o_ps", bufs=1)
                    attn_chunk(1, qc, range(KT // 2), DVE_KTS[(1, qc)])
                    norm_q.append((1, qc))
                    out_sts = list(range(qc * 4, qc * 4 + 4))

                # tail: last normalize in 128-col units, each output
                # s-tile emitted as soon as its At columns are ready
                j, qc = norm_q.pop(0)
                o_l = o_ps[(j, qc)]
                q0 = qc * 512
                oc = dnp.tile([65, 1024], F32, tag="oc", name="oc", bufs=2)
                bc = dnp.tile([64, 1024], F32, tag="bc", name="bc", bufs=2)
                nc.vector.reciprocal(bc[0:1, :], o_l[64:65, :])
                nc.scalar.activation(oc, o_l, AF.Copy)
                nc.gpsimd.partition_broadcast(bc, bc[0:1, :])
                for u in range(4):
                    for hi in range(2):
                        nc.gpsimd.tensor_tensor(
                            At[j][64 * hi:64 * hi + 64,
                                  q0 + u * 128:q0 + (u + 1) * 128],
                            oc[0:64, hi * 512 + u * 128:
                               hi * 512 + (u + 1) * 128],
                            bc[:, hi * 512 + u * 128:
                               hi * 512 + (u + 1) * 128], op=OP.mult)
                    if u < 3:
                        emit_out_st(out_sts[u], tail=(u % 2 == 0))
                    else:
                        # final tile: split drain+DMA in halves so the
                        # last DMA starts as early as possible
                        st = out_sts[u]
                        op_ps = psP(512)
                        for r in range(2):
                            nc.tensor.matmul(
                                op_ps, At[r][:, st * 128:(st + 1) * 128],
                                wo_t[r], start=(r == 0), stop=(r == 1))
                        ot = osb.tile([128, D], F32, tag="ot", name="ot")
                        for h in range(2):
                            cs = slice(h * 256, (h + 1) * 256)
                            nc.vector.tensor_tensor(
                                ot[:, cs], op_ps[:, cs], bo_bc[:, cs],
                                op=OP.add)
                            nc.sync.dma_start(
                                out[st * 128:(st + 1) * 128, cs],
                                ot[:, cs])

    nc.compile()
    return nc


_NC = None


def kernel(mod1_feat, mod2_feat, Wq, bq, Wk, bk, Wv, bv, Wo, bo, Ws, bs,
           Wsh, bsh):
    global _NC
    import ml_dtypes
    if _NC is None:
        _NC = build()
    bf = ml_dtypes.bfloat16
    zeros_bo = np.zeros_like(bo)
    x1b = [np.ascontiguousarray(mod1_feat[b].T).astype(bf)
           for b in range(B)]
    x2b = [np.ascontiguousarray(mod2_feat[b].T).astype(bf)
           for b in range(B)]
    in_maps = []
    for c in range(N_CORES):
        b, g = c // 2, c % 2
        cols = slice(g * NG, (g + 1) * NG)
        ball = np.ascontiguousarray(np.concatenate(
            [bs[cols], bk[cols], bsh[cols], bv[cols], bq[cols]]
        ).astype(np.float32))
        in_maps.append({
            "x1": x1b[b],
            "x2": x2b[b],
            "wq": np.ascontiguousarray(Wq[:, cols]).astype(bf),
            "wk": np.ascontiguousarray(Wk[:, cols]).astype(bf),
            "wv": np.ascontiguousarray(Wv[:, cols]).astype(bf),
            "ws": np.ascontiguousarray(Ws[:, cols]).astype(bf),
            "wsh": np.ascontiguousarray(Wsh[:, cols]).astype(bf),
            "ball": ball,
            "wo": np.ascontiguousarray(Wo[cols, :]),
            "bo": bo if g == 0 else zeros_bo,
        })
    res = run_bass_kernel_spmd(_NC, in_maps, list(range(N_CORES)))
    outs = [res.results[c]["out"] for c in range(N_CORES)]
    full = np.stack([outs[2 * b] + outs[2 * b + 1] for b in range(B)])
    return full.astype(np.float32)
